# revision 11
# baseline (speedup 1.0000x reference)
"""DigitCaps (CapsNet dynamic routing) Trainium2 kernel, v2.

Math (per reference):
  u_hat[b,i,o,d] = sum_k W[i,o,d,k] * x[b,i,k]      B=256, IC=1152, K=8, O=10, D=16
  3 routing iters: c = softmax_o(bl); s = sum_i c*u_hat; v = squash(s);
                   bl += sum_d u_hat*v
  out v: [B, 10, 16]

Data-parallel over batch: 8 cores x 32 samples, 4 bgroups of 8 per core.
Einsum on TensorE with block-diagonal x (lhsT stationary, wr moving).
v2 changes vs baseline:
  - iter-0 s is a DENSE matmul: c uniform -> s0 = 0.1*sum_i u_hat
    = accumulation of x2d[g].T @ wr[g] over all 72 groups (out [32,160]).
    Removes 288 routing matmuls + 4 diag extracts; v0 broadcast to
    (i16,b8) rows via a tiny select matmul per bgroup.
  - bl-update tree: 3 chunks of 24 groups (FD 3840) instead of 8 chunks
    of 9; last tree level writes/accumulates bl directly.
  - call build split: b' 0:4 on VectorE (one broadcast TT), b' 4:7 on
    ScalarE (per-partition-scalar masked copies).
  - softmax: cC mult outputs bf16 directly (cast folded).
  - fewer, larger DMAs.
"""

import sys

sys.path.insert(0, "/opt/trn_rl_repo")

import numpy as np
import ml_dtypes

import concourse.bass as bass
import concourse.bacc as bacc_mod
from concourse import mybir
from concourse.tile import TileContext
from concourse.bass_utils import run_bass_kernel_spmd

BF16 = ml_dtypes.bfloat16

# Problem dims (hardcoded per harness contract)
B, IC, KD, OC, OD = 256, 1152, 8, 10, 16
NCORES = 8
BL = B // NCORES          # 32 samples per core
BG = 8                    # bgroup size
NBG = BL // BG            # 4 bgroups
G = IC // 16              # 72 groups of 16 in-caps
ODF = OC * OD             # 160
ITERS = 3
GO = G * OC               # 720 logit columns
ZCH = 24                  # g-chunk size for the bl-update pipeline
NZCH = G // ZCH           # 3 chunks
XCH = 18                  # g-chunk size for xblk DMA (4 chunks/bg)
NXCH = G // XCH

_BUILT = None


def _consts():
    """Host-side constant tensors shared by all cores."""
    p = np.arange(128)
    bb_of_p = p % 8  # b-lane of partition (i_sub,b)

    # mcb [128, 80] bf16: delta(b(p) == b') at column (b'*10+o)
    col_b = (np.arange(80) // 10)
    mcb = (bb_of_p[:, None] == col_b[None, :]).astype(np.float32)

    # msks [80, 160] f32: delta(o == o') ; row (b,o), col (o'*16+d)
    row_o = np.arange(80) % 10
    col_o = np.arange(160) // 16
    msks = (row_o[:, None] == col_o[None, :]).astype(np.float32)

    # arep [80, 128] bf16: delta(b == b') ; row (b,o), col (i_sub*8+b')
    row_b = np.arange(80) // 10
    col_b2 = np.arange(128) % 8
    arep = (row_b[:, None] == col_b2[None, :]).astype(np.float32)

    # sel [32, 512] bf16: sel[b, bg*128 + i_sub*8 + b8] = (b == bg*8+b8)
    sel = np.zeros((32, 4, 16, 8), np.float32)
    for bg in range(NBG):
        for b8 in range(BG):
            sel[bg * BG + b8, bg, :, b8] = 1.0
    sel = sel.reshape(32, 512)

    return {
        "mcb": mcb.astype(BF16),
        "mcf": mcb,  # f32 copy for per-partition scalar masks
        "msks": msks,
        "arep": arep.astype(BF16),
        "sel": sel.astype(BF16),
    }


def _prep_core(x_c):
    """Per-core input prep. x_c: [32, 1152, 8] f32.
    xblk [NBG, 128, G*128] bf16 block-diagonal:
      xblk[bg, i_sub*8+k, g*128 + i_sub*8+b] = x_c[bg*8+b, g*16+i_sub, k]
    x2d [128, G*32] bf16 dense: x2d[i_sub*8+k, g*32+b] = x_c[b, g*16+i_sub, k]
    """
    xblk = np.zeros((NBG, 128, G * 128), np.float32)
    xv = x_c.reshape(NBG, BG, G, 16, KD)  # [bg, b, g, i_sub, k]
    for i_sub in range(16):
        blk = xv[:, :, :, i_sub, :].transpose(0, 3, 2, 1)  # [bg, k, g, b]
        xblk[:, i_sub * 8 : i_sub * 8 + 8, :].reshape(NBG, 8, G, 128)[
            :, :, :, i_sub * 8 : i_sub * 8 + 8
        ] = blk
    x2 = x_c.reshape(32, G, 16, KD).transpose(2, 3, 1, 0)  # [i_sub, k, g, b]
    x2d = np.ascontiguousarray(x2.reshape(128, G * 32))
    return {"xblk": xblk.astype(BF16), "x2d": x2d.astype(BF16)}


def _prep_w(W):
    """wr [128, G*160] bf16: wr[i_sub*8+k, g*160 + o*16+d] = W[g*16+i_sub,o,d,k]"""
    wv = W.reshape(G, 16, OC, OD, KD)  # [g, i_sub, o, d, k]
    wr = wv.transpose(1, 4, 0, 2, 3).reshape(128, G * ODF)
    return np.ascontiguousarray(wr).astype(BF16)


def _in_maps(x, W):
    x = np.asarray(x, np.float32)
    W = np.asarray(W, np.float32)
    wr = _prep_w(W)
    cst = _consts()
    in_maps = []
    for c in range(NCORES):
        m = _prep_core(x[c * BL : (c + 1) * BL])
        m["wr"] = wr
        m.update(cst)
        in_maps.append(m)
    return in_maps


def _build():
    global _BUILT
    if _BUILT is not None:
        return _BUILT

    nc = bacc_mod.Bacc()
    dt = mybir.dt
    xblk_d = nc.dram_tensor("xblk", [NBG, 128, G * 128], dt.bfloat16, kind="ExternalInput")
    x2d_d = nc.dram_tensor("x2d", [128, G * 32], dt.bfloat16, kind="ExternalInput")
    wr_d = nc.dram_tensor("wr", [128, G * ODF], dt.bfloat16, kind="ExternalInput")
    mcb_d = nc.dram_tensor("mcb", [128, 80], dt.bfloat16, kind="ExternalInput")
    mcf_d = nc.dram_tensor("mcf", [128, 80], dt.float32, kind="ExternalInput")
    msks_d = nc.dram_tensor("msks", [80, ODF], dt.float32, kind="ExternalInput")
    arep_d = nc.dram_tensor("arep", [80, 128], dt.bfloat16, kind="ExternalInput")
    sel_d = nc.dram_tensor("sel", [32, 512], dt.bfloat16, kind="ExternalInput")
    vout_d = nc.dram_tensor("vout", [BL, OC, OD], dt.float32, kind="ExternalOutput")

    AF = mybir.ActivationFunctionType
    ALU = mybir.AluOpType
    AX = mybir.AxisListType

    with TileContext(nc) as tc:
        with (
            tc.tile_pool(name="consts", bufs=1) as cpool,
            tc.tile_pool(name="wrp", bufs=1) as wpool,
            tc.tile_pool(name="xbp", bufs=3) as xpool,
            tc.tile_pool(name="uhp", bufs=1) as uhpool,
            tc.tile_pool(name="blp", bufs=1) as blpool,
            tc.tile_pool(name="route", bufs=2) as rpool,
            tc.tile_pool(name="ztmp", bufs=1) as zpool,
            tc.tile_pool(name="small", bufs=2) as spool,
            tc.tile_pool(name="vr", bufs=2) as vpool,
            tc.tile_pool(name="pe", bufs=2, space="PSUM") as pe_pool,
            tc.tile_pool(name="ps", bufs=2, space="PSUM") as ps_pool,
            tc.tile_pool(name="pv", bufs=1, space="PSUM") as pv_pool,
            tc.tile_pool(name="p0", bufs=1, space="PSUM") as p0_pool,
        ):
            # ---- resident constants / weights
            wr_sb = wpool.tile([128, G * ODF], dt.bfloat16, tag="wr")
            for s in range(6):
                w = G * ODF // 6
                nc.sync.dma_start(
                    out=wr_sb[:, s * w : (s + 1) * w],
                    in_=wr_d[:, s * w : (s + 1) * w],
                )
            x2d = wpool.tile([128, G * 32], dt.bfloat16, tag="x2d")
            nc.sync.dma_start(out=x2d[:], in_=x2d_d[:])
            mcb = cpool.tile([128, 80], dt.bfloat16, tag="mcb")
            nc.sync.dma_start(out=mcb[:], in_=mcb_d[:])
            mcf = cpool.tile([128, 80], dt.float32, tag="mcf")
            nc.sync.dma_start(out=mcf[:], in_=mcf_d[:])
            msks = cpool.tile([80, ODF], dt.float32, tag="msks")
            nc.sync.dma_start(out=msks[:], in_=msks_d[:])
            arep = cpool.tile([80, 128], dt.bfloat16, tag="arep")
            nc.sync.dma_start(out=arep[:], in_=arep_d[:])
            sel = cpool.tile([32, 512], dt.bfloat16, tag="sel")
            nc.sync.dma_start(out=sel[:], in_=sel_d[:])
            czero = cpool.tile([128, 1], dt.float32, tag="czero")
            nc.vector.memset(czero[:], 0.0)
            ceps = cpool.tile([80, 1], dt.float32, tag="ceps")
            nc.vector.memset(ceps[:], 1e-8)

            # ---- s0 = 0.1 * sum_i u_hat  (dense accumulation, all 32 b)
            ps0 = p0_pool.tile([32, ODF], dt.float32, tag="ps0")
            for g in range(G):
                nc.tensor.matmul(
                    ps0[:],
                    x2d[:, g * 32 : (g + 1) * 32],
                    wr_sb[:, g * ODF : (g + 1) * ODF],
                    start=(g == 0),
                    stop=(g == G - 1),
                )
            # squash on [32, ...]: v0 = fac*s0, fac = ns/((1+ns)sqrt(ns+eps)),
            # s0 = 0.1*T (T = ps0); ns from Square(0.1*T).
            sq0 = spool.tile([32, ODF], dt.float32, tag="sq0")
            nc.scalar.activation(
                out=sq0[:], in_=ps0[:], func=AF.Square, bias=czero[:32], scale=0.1
            )
            ns0 = spool.tile([32, OC], dt.float32, tag="ns0")
            nc.vector.tensor_reduce(
                out=ns0[:],
                in_=sq0[:].rearrange("p (o d) -> p o d", o=OC),
                axis=AX.X,
                op=ALU.add,
            )
            sqn0 = spool.tile([32, OC], dt.float32, tag="sqn0")
            nc.scalar.activation(
                out=sqn0[:], in_=ns0[:], func=AF.Sqrt, bias=ceps[:32]
            )
            den0 = spool.tile([32, OC], dt.float32, tag="den0")
            nc.vector.scalar_tensor_tensor(
                out=den0[:], in0=ns0[:], scalar=1.0, in1=sqn0[:],
                op0=ALU.add, op1=ALU.mult,
            )
            rden0 = spool.tile([32, OC], dt.float32, tag="rden0")
            nc.vector.reciprocal(out=rden0[:], in_=den0[:])
            fac0 = spool.tile([32, OC], dt.float32, tag="fac0")
            # fac = 0.1 * ns * rden  (0.1 for s0 = 0.1*T)
            nc.vector.tensor_tensor(
                out=fac0[:], in0=ns0[:], in1=rden0[:], op=ALU.mult
            )
            nc.vector.tensor_scalar_mul(fac0[:], fac0[:], 0.1)
            v0_bf = spool.tile([32, ODF], dt.bfloat16, tag="v0_bf")
            nc.vector.tensor_tensor(
                out=v0_bf[:].rearrange("p (o d) -> p o d", o=OC),
                in0=ps0[:].rearrange("p (o d) -> p o d", o=OC),
                in1=fac0[:].unsqueeze(2).broadcast_to([32, OC, OD]),
                op=ALU.mult,
            )
            # vrep0 per bgroup via select matmul
            vrep0_t = []
            for bg in range(NBG):
                pv = pv_pool.tile([128, ODF], dt.float32, tag="pv")
                nc.tensor.matmul(
                    pv[:], sel[:, bg * 128 : (bg + 1) * 128], v0_bf[:],
                    start=True, stop=True,
                )
                vr = vpool.tile([128, ODF], dt.bfloat16, tag=f"vrep0{bg}")
                nc.scalar.copy(out=vr[:], in_=pv[:])
                vrep0_t.append(vr)

            # ---- Phase A: einsum for all bgroups
            uh_t = []
            bl_t = []
            for bg in range(NBG):
                uh = uhpool.tile([128, G * ODF], dt.bfloat16, tag=f"uh{bg}")
                uh_t.append(uh)
                bl = blpool.tile([128, GO], dt.float32, tag=f"bl{bg}")
                bl_t.append(bl)

            def einsum_bg(bg):
                uh = uh_t[bg]
                for xc in range(NXCH):
                    xt = xpool.tile([128, XCH * 128], dt.bfloat16, tag="xt")
                    nc.sync.dma_start(
                        out=xt[:],
                        in_=xblk_d[bg][:, xc * XCH * 128 : (xc + 1) * XCH * 128],
                    )
                    for t in range(XCH // 6):
                        pe = pe_pool.tile([128, 960], dt.float32, tag="pe")
                        for j in range(6):
                            gl = t * 6 + j           # local g in chunk
                            g = xc * XCH + gl        # global g
                            nc.tensor.matmul(
                                pe[:, j * ODF : (j + 1) * ODF],
                                xt[:, gl * 128 : (gl + 1) * 128],
                                wr_sb[:, g * ODF : (g + 1) * ODF],
                                start=True,
                                stop=True,
                            )
                        g0 = xc * XCH + t * 6
                        nc.scalar.copy(
                            out=uh[:, g0 * ODF : (g0 + 6) * ODF], in_=pe[:]
                        )

            GPS_TAIL = True  # run t2/t1/bl-add tree tail on GpSimd

            def bl_update(bg, vrep, first):
                """bl[bg] (+)= sum_d uh[bg]*vrep ; first=True writes fresh."""
                uh = uh_t[bg]
                bl = bl_t[bg]
                tail = nc.gpsimd if GPS_TAIL else nc.vector
                for ch in range(NZCH):
                    cs = ch * ZCH
                    z = zpool.tile([128, ZCH * ODF], dt.bfloat16, tag="z")
                    nc.vector.tensor_tensor(
                        out=z[:].rearrange("p (g f) -> p g f", f=ODF),
                        in0=uh[:, cs * ODF : (cs + ZCH) * ODF].rearrange(
                            "p (g f) -> p g f", f=ODF
                        ),
                        in1=vrep[:].unsqueeze(1).broadcast_to([128, ZCH, ODF]),
                        op=ALU.mult,
                    )
                    t8 = zpool.tile([128, ZCH * 80], dt.bfloat16, tag="t8")
                    zv = z[:].rearrange("p (g o d) -> p g o d", o=OC, d=OD)
                    nc.vector.tensor_tensor(
                        out=t8[:].rearrange("p (g o d) -> p g o d", o=OC, d=8),
                        in0=zv[:, :, :, 0:8],
                        in1=zv[:, :, :, 8:16],
                        op=ALU.add,
                    )
                    t4 = zpool.tile([128, ZCH * 40], dt.bfloat16, tag="t4")
                    t8v = t8[:].rearrange("p (g o d) -> p g o d", o=OC, d=8)
                    nc.vector.tensor_tensor(
                        out=t4[:].rearrange("p (g o d) -> p g o d", o=OC, d=4),
                        in0=t8v[:, :, :, 0:4],
                        in1=t8v[:, :, :, 4:8],
                        op=ALU.add,
                    )
                    t2 = zpool.tile([128, ZCH * 20], dt.bfloat16, tag="t2")
                    t4v = t4[:].rearrange("p (g o d) -> p g o d", o=OC, d=4)
                    tail.tensor_tensor(
                        out=t2[:].rearrange("p (g o d) -> p g o d", o=OC, d=2),
                        in0=t4v[:, :, :, 0:2],
                        in1=t4v[:, :, :, 2:4],
                        op=ALU.add,
                    )
                    t2v = t2[:].rearrange("p (g o d) -> p g o d", o=OC, d=2)
                    blv = bl[:, cs * OC : (cs + ZCH) * OC]
                    if first:
                        tail.tensor_tensor(
                            out=blv.rearrange("p (g o) -> p g o", o=OC).unsqueeze(3),
                            in0=t2v[:, :, :, 0:1],
                            in1=t2v[:, :, :, 1:2],
                            op=ALU.add,
                        )
                    else:
                        t1 = zpool.tile([128, ZCH * OC], dt.bfloat16, tag="t1")
                        tail.tensor_tensor(
                            out=t1[:].rearrange("p (g o) -> p g o", o=OC).unsqueeze(3),
                            in0=t2v[:, :, :, 0:1],
                            in1=t2v[:, :, :, 1:2],
                            op=ALU.add,
                        )
                        tail.tensor_tensor(
                            out=blv, in0=blv, in1=t1[:], op=ALU.add
                        )

            # ---- iter-0: einsum interleaved with bl updates (software pipeline)
            einsum_bg(0)
            einsum_bg(1)
            bl_update(0, vrep0_t[0], first=True)
            einsum_bg(2)
            bl_update(1, vrep0_t[1], first=True)
            einsum_bg(3)
            bl_update(2, vrep0_t[2], first=True)
            bl_update(3, vrep0_t[3], first=True)

            # ---- iters 1,2: stages, software-pipelined across (it,bg) columns
            col_state = {}

            def stage1(it, bg):
                """softmax over o + call build."""
                bl = bl_t[bg]
                ee = rpool.tile([128, GO], dt.float32, tag="ee")
                nc.scalar.activation(
                    out=ee[:], in_=bl[:], func=AF.Exp, bias=czero[:]
                )
                zz = rpool.tile([128, G], dt.float32, tag="zz")
                nc.vector.tensor_reduce(
                    out=zz[:],
                    in_=ee[:].rearrange("p (g o) -> p g o", o=OC),
                    axis=AX.X,
                    op=ALU.add,
                )
                rz = rpool.tile([128, G], dt.float32, tag="rz")
                nc.vector.reciprocal(out=rz[:], in_=zz[:])
                cCb = rpool.tile([128, GO], dt.bfloat16, tag="cCb")
                nc.vector.tensor_tensor(
                    out=cCb[:].rearrange("p (g o) -> p g o", o=OC),
                    in0=ee[:].rearrange("p (g o) -> p g o", o=OC),
                    in1=rz[:].unsqueeze(2).broadcast_to([128, G, OC]),
                    op=ALU.mult,
                )
                call = rpool.tile([128, G * 80], dt.bfloat16, tag="call")
                callv = call[:].rearrange(
                    "p (g b o) -> p g b o", b=BG, o=OC
                )
                # b' 0:4 on VectorE (one broadcast tensor_tensor)
                nc.vector.tensor_tensor(
                    out=callv[:, :, 0:4, :],
                    in0=cCb[:]
                    .rearrange("p (g o) -> p g o", o=OC)
                    .unsqueeze(2)
                    .broadcast_to([128, G, 4, OC]),
                    in1=mcb[:, 0:40]
                    .rearrange("p (b o) -> p b o", o=OC)
                    .unsqueeze(1)
                    .broadcast_to([128, G, 4, OC]),
                    op=ALU.mult,
                )
                # b' 4:8 on ScalarE (masked copies, per-partition scale)
                for bp in range(4, BG):
                    nc.scalar.activation(
                        out=callv[:, :, bp, :],
                        in_=cCb[:].rearrange("p (g o) -> p g o", o=OC),
                        func=AF.Copy,
                        bias=0.0,
                        scale=mcf[:, bp * OC : bp * OC + 1],
                    )
                col_state[(it, bg)] = {"call": call}

            def stage2(it, bg):
                """s matmul: accumulate over all 72 groups."""
                call = col_state[(it, bg)]["call"]
                uh = uh_t[bg]
                ps = ps_pool.tile([80, ODF], dt.float32, tag="ps")
                for g in range(G):
                    nc.tensor.matmul(
                        ps[:],
                        call[:, g * 80 : (g + 1) * 80],
                        uh[:, g * ODF : (g + 1) * ODF],
                        start=(g == 0),
                        stop=(g == G - 1),
                    )
                col_state[(it, bg)]["ps"] = ps

            def stage3(it, bg):
                """diag extract + squash; it1: build vrep + bl update input,
                it2: final v -> DRAM."""
                ps = col_state[(it, bg)]["ps"]
                # extract diag o==o' -> s_t [80,16]
                tmp = spool.tile([80, ODF], dt.float32, tag="tmp")
                nc.vector.tensor_tensor(
                    out=tmp[:].rearrange("p (d o) -> p d o", o=OC),
                    in0=ps[:]
                    .rearrange("p (o d) -> p o d", o=OC)
                    .transpose([0, 2, 1]),
                    in1=msks[:]
                    .rearrange("p (o d) -> p o d", o=OC)
                    .transpose([0, 2, 1]),
                    op=ALU.mult,
                )
                s_t = spool.tile([80, OD], dt.float32, tag="s_t")
                nc.vector.tensor_reduce(
                    out=s_t[:],
                    in_=tmp[:].rearrange("p (d o) -> p d o", o=OC),
                    axis=AX.X,
                    op=ALU.add,
                )

                # squash: fac = ns / ((1+ns) * sqrt(ns+eps))
                sq = spool.tile([80, OD], dt.float32, tag="sq")
                ns = spool.tile([80, 1], dt.float32, tag="ns")
                nc.scalar.activation(
                    out=sq[:], in_=s_t[:], func=AF.Square, bias=czero[:80]
                )
                nc.vector.tensor_reduce(
                    out=ns[:], in_=sq[:], axis=AX.X, op=ALU.add
                )
                sqn = spool.tile([80, 1], dt.float32, tag="sqn")
                nc.scalar.activation(
                    out=sqn[:], in_=ns[:], func=AF.Sqrt, bias=ceps[:]
                )
                den = spool.tile([80, 1], dt.float32, tag="den")
                nc.vector.scalar_tensor_tensor(
                    out=den[:], in0=ns[:], scalar=1.0, in1=sqn[:],
                    op0=ALU.add, op1=ALU.mult,
                )
                rden = spool.tile([80, 1], dt.float32, tag="rden")
                nc.vector.reciprocal(out=rden[:], in_=den[:])
                fac = spool.tile([80, 1], dt.float32, tag="fac")
                nc.vector.tensor_tensor(
                    out=fac[:], in0=ns[:], in1=rden[:], op=ALU.mult
                )

                if it == ITERS - 1:
                    v_f = spool.tile([80, OD], dt.float32, tag="v_f")
                    nc.vector.tensor_scalar_mul(v_f[:], s_t[:], fac[:])
                    nc.sync.dma_start(
                        out=vout_d[bg * BG : (bg + 1) * BG].rearrange(
                            "b o d -> (b o) d"
                        ),
                        in_=v_f[:],
                    )
                    return

                v_bf = spool.tile([80, OD], dt.bfloat16, tag="v_bf")
                nc.vector.tensor_scalar_mul(v_bf[:], s_t[:], fac[:])

                # vexp[(b,o),(o',d)] = v[b,o,d] * delta(o==o')
                vexp = spool.tile([80, ODF], dt.bfloat16, tag="vexp")
                nc.vector.tensor_tensor(
                    out=vexp[:].rearrange("p (o d) -> p o d", o=OC),
                    in0=msks[:].rearrange("p (o d) -> p o d", o=OC),
                    in1=v_bf[:].unsqueeze(1).broadcast_to([80, OC, OD]),
                    op=ALU.mult,
                )
                pv = pv_pool.tile([128, ODF], dt.float32, tag="pv")
                nc.tensor.matmul(pv[:], arep[:], vexp[:], start=True, stop=True)
                vrep = vpool.tile([128, ODF], dt.bfloat16, tag="vrep")
                nc.scalar.copy(out=vrep[:], in_=pv[:])
                col_state[(it, bg)]["vrep"] = vrep

            # pipelined emission: stage1(n) | stage3(n-1) | stage2(n) |
            # bl_update(n-1, it1 only)
            cols = [(1, 0), (1, 1), (1, 2), (1, 3), (2, 0), (2, 1), (2, 2), (2, 3)]
            for n in range(len(cols)):
                stage1(*cols[n])
                if n >= 1:
                    stage3(*cols[n - 1])
                stage2(*cols[n])
                if n >= 1 and cols[n - 1][0] == 1:
                    bl_update(cols[n - 1][1], col_state[cols[n - 1]]["vrep"],
                              first=False)
            stage3(*cols[-1])

    nc.finalize()
    _BUILT = nc
    return nc


_WARMED = False


def kernel(x, W):
    global _WARMED
    nc = _build()
    in_maps = _in_maps(x, W)
    if not _WARMED:
        # First execution after an in-process compile can return a
        # partially-unwritten output buffer (observed: bgroup 0 rows NaN).
        # Run once with the real inputs and discard.
        run_bass_kernel_spmd(nc, in_maps, core_ids=list(range(NCORES)))
        _WARMED = True
    res = run_bass_kernel_spmd(nc, in_maps, core_ids=list(range(NCORES)))
    outs = res.results
    v = np.concatenate([np.asarray(o["vout"]) for o in outs], axis=0)
    return v.astype(np.float32)


if __name__ == "__main__":
    rng = np.random.default_rng(0)
    x = rng.standard_normal((B, IC, KD), np.float32)
    W = rng.standard_normal((IC, OC, OD, KD), np.float32)
    v = kernel(x, W)
    print("out", v.shape, v.dtype, float(np.abs(v).mean()))


# revision 12
# speedup vs baseline: 1.0593x; 1.0593x over previous
"""DigitCaps (CapsNet dynamic routing) Trainium2 kernel, v2.

Math (per reference):
  u_hat[b,i,o,d] = sum_k W[i,o,d,k] * x[b,i,k]      B=256, IC=1152, K=8, O=10, D=16
  3 routing iters: c = softmax_o(bl); s = sum_i c*u_hat; v = squash(s);
                   bl += sum_d u_hat*v
  out v: [B, 10, 16]

Data-parallel over batch: 8 cores x 32 samples, 4 bgroups of 8 per core.
Einsum on TensorE with block-diagonal x (lhsT stationary, wr moving).
v2 changes vs baseline:
  - iter-0 s is a DENSE matmul: c uniform -> s0 = 0.1*sum_i u_hat
    = accumulation of x2d[g].T @ wr[g] over all 72 groups (out [32,160]).
    Removes 288 routing matmuls + 4 diag extracts; v0 broadcast to
    (i16,b8) rows via a tiny select matmul per bgroup.
  - bl-update tree: 3 chunks of 24 groups (FD 3840) instead of 8 chunks
    of 9; last tree level writes/accumulates bl directly.
  - call build split: b' 0:4 on VectorE (one broadcast TT), b' 4:7 on
    ScalarE (per-partition-scalar masked copies).
  - softmax: cC mult outputs bf16 directly (cast folded).
  - fewer, larger DMAs.
"""

import sys

sys.path.insert(0, "/opt/trn_rl_repo")

import numpy as np
import ml_dtypes

import concourse.bass as bass
import concourse.bacc as bacc_mod
from concourse import mybir
from concourse.tile import TileContext
from concourse.bass_utils import run_bass_kernel_spmd

BF16 = ml_dtypes.bfloat16

# Problem dims (hardcoded per harness contract)
B, IC, KD, OC, OD = 256, 1152, 8, 10, 16
NCORES = 8
BL = B // NCORES          # 32 samples per core
BG = 8                    # bgroup size
NBG = BL // BG            # 4 bgroups
G = IC // 16              # 72 groups of 16 in-caps
ODF = OC * OD             # 160
ITERS = 3
GO = G * OC               # 720 logit columns
ZCH = 24                  # g-chunk size for the bl-update pipeline
NZCH = G // ZCH           # 3 chunks
XCH = 18                  # g-chunk size for xblk DMA (4 chunks/bg)
NXCH = G // XCH

_BUILT = None


def _consts():
    """Host-side constant tensors shared by all cores."""
    p = np.arange(128)
    bb_of_p = p % 8  # b-lane of partition (i_sub,b)

    # mcb [128, 80] bf16: delta(b(p) == b') at column (b'*10+o)
    col_b = (np.arange(80) // 10)
    mcb = (bb_of_p[:, None] == col_b[None, :]).astype(np.float32)

    # msks [80, 160] f32: delta(o == o') ; row (b,o), col (o'*16+d)
    row_o = np.arange(80) % 10
    col_o = np.arange(160) // 16
    msks = (row_o[:, None] == col_o[None, :]).astype(np.float32)

    # arep [80, 128] bf16: delta(b == b') ; row (b,o), col (i_sub*8+b')
    row_b = np.arange(80) // 10
    col_b2 = np.arange(128) % 8
    arep = (row_b[:, None] == col_b2[None, :]).astype(np.float32)

    # sel [32, 512] bf16: sel[b, bg*128 + i_sub*8 + b8] = (b == bg*8+b8)
    sel = np.zeros((32, 4, 16, 8), np.float32)
    for bg in range(NBG):
        for b8 in range(BG):
            sel[bg * BG + b8, bg, :, b8] = 1.0
    sel = sel.reshape(32, 512)

    return {
        "mcb": mcb.astype(BF16),
        "mcf": mcb,  # f32 copy for per-partition scalar masks
        "msks": msks,
        "arep": arep.astype(BF16),
        "sel": sel.astype(BF16),
    }


def _prep_core(x_c):
    """Per-core input prep. x_c: [32, 1152, 8] f32.
    xblk [NBG, 128, G*128] bf16 block-diagonal:
      xblk[bg, i_sub*8+k, g*128 + i_sub*8+b] = x_c[bg*8+b, g*16+i_sub, k]
    x2d [128, G*32] bf16 dense: x2d[i_sub*8+k, g*32+b] = x_c[b, g*16+i_sub, k]
    """
    xblk = np.zeros((NBG, 128, G * 128), np.float32)
    xv = x_c.reshape(NBG, BG, G, 16, KD)  # [bg, b, g, i_sub, k]
    for i_sub in range(16):
        blk = xv[:, :, :, i_sub, :].transpose(0, 3, 2, 1)  # [bg, k, g, b]
        xblk[:, i_sub * 8 : i_sub * 8 + 8, :].reshape(NBG, 8, G, 128)[
            :, :, :, i_sub * 8 : i_sub * 8 + 8
        ] = blk
    x2 = x_c.reshape(32, G, 16, KD).transpose(2, 3, 1, 0)  # [i_sub, k, g, b]
    x2d = np.ascontiguousarray(x2.reshape(128, G * 32))
    return {"xblk": xblk.astype(BF16), "x2d": x2d.astype(BF16)}


def _prep_w(W):
    """wr [128, G*160] bf16: wr[i_sub*8+k, g*160 + o*16+d] = W[g*16+i_sub,o,d,k]"""
    wv = W.reshape(G, 16, OC, OD, KD)  # [g, i_sub, o, d, k]
    wr = wv.transpose(1, 4, 0, 2, 3).reshape(128, G * ODF)
    return np.ascontiguousarray(wr).astype(BF16)


def _in_maps(x, W):
    x = np.asarray(x, np.float32)
    W = np.asarray(W, np.float32)
    wr = _prep_w(W)
    cst = _consts()
    in_maps = []
    for c in range(NCORES):
        m = _prep_core(x[c * BL : (c + 1) * BL])
        m["wr"] = wr
        m.update(cst)
        in_maps.append(m)
    return in_maps


def _build():
    global _BUILT
    if _BUILT is not None:
        return _BUILT

    nc = bacc_mod.Bacc()
    dt = mybir.dt
    xblk_d = nc.dram_tensor("xblk", [NBG, 128, G * 128], dt.bfloat16, kind="ExternalInput")
    x2d_d = nc.dram_tensor("x2d", [128, G * 32], dt.bfloat16, kind="ExternalInput")
    wr_d = nc.dram_tensor("wr", [128, G * ODF], dt.bfloat16, kind="ExternalInput")
    mcb_d = nc.dram_tensor("mcb", [128, 80], dt.bfloat16, kind="ExternalInput")
    mcf_d = nc.dram_tensor("mcf", [128, 80], dt.float32, kind="ExternalInput")
    msks_d = nc.dram_tensor("msks", [80, ODF], dt.float32, kind="ExternalInput")
    arep_d = nc.dram_tensor("arep", [80, 128], dt.bfloat16, kind="ExternalInput")
    sel_d = nc.dram_tensor("sel", [32, 512], dt.bfloat16, kind="ExternalInput")
    vout_d = nc.dram_tensor("vout", [BL, OC, OD], dt.float32, kind="ExternalOutput")

    AF = mybir.ActivationFunctionType
    ALU = mybir.AluOpType
    AX = mybir.AxisListType

    with TileContext(nc) as tc:
        with (
            tc.tile_pool(name="consts", bufs=1) as cpool,
            tc.tile_pool(name="wrp", bufs=1) as wpool,
            tc.tile_pool(name="xbp", bufs=3) as xpool,
            tc.tile_pool(name="uhp", bufs=1) as uhpool,
            tc.tile_pool(name="blp", bufs=1) as blpool,
            tc.tile_pool(name="route", bufs=2) as rpool,
            tc.tile_pool(name="ztmp", bufs=1) as zpool,
            tc.tile_pool(name="small", bufs=2) as spool,
            tc.tile_pool(name="vr", bufs=2) as vpool,
            tc.tile_pool(name="pe", bufs=2, space="PSUM") as pe_pool,
            tc.tile_pool(name="ps", bufs=2, space="PSUM") as ps_pool,
            tc.tile_pool(name="pv", bufs=1, space="PSUM") as pv_pool,
            tc.tile_pool(name="p0", bufs=1, space="PSUM") as p0_pool,
        ):
            # ---- resident constants / weights
            wr_sb = wpool.tile([128, G * ODF], dt.bfloat16, tag="wr")
            for s in range(6):
                w = G * ODF // 6
                nc.sync.dma_start(
                    out=wr_sb[:, s * w : (s + 1) * w],
                    in_=wr_d[:, s * w : (s + 1) * w],
                )
            x2d = wpool.tile([128, G * 32], dt.bfloat16, tag="x2d")
            nc.sync.dma_start(out=x2d[:], in_=x2d_d[:])
            mcb = cpool.tile([128, 80], dt.bfloat16, tag="mcb")
            nc.sync.dma_start(out=mcb[:], in_=mcb_d[:])
            mcf = cpool.tile([128, 80], dt.float32, tag="mcf")
            nc.sync.dma_start(out=mcf[:], in_=mcf_d[:])
            msks = cpool.tile([80, ODF], dt.float32, tag="msks")
            nc.sync.dma_start(out=msks[:], in_=msks_d[:])
            arep = cpool.tile([80, 128], dt.bfloat16, tag="arep")
            nc.sync.dma_start(out=arep[:], in_=arep_d[:])
            sel = cpool.tile([32, 512], dt.bfloat16, tag="sel")
            nc.sync.dma_start(out=sel[:], in_=sel_d[:])
            czero = cpool.tile([128, 1], dt.float32, tag="czero")
            nc.vector.memset(czero[:], 0.0)
            ceps = cpool.tile([80, 1], dt.float32, tag="ceps")
            nc.vector.memset(ceps[:], 1e-8)

            # ---- s0 = 0.1 * sum_i u_hat  (dense accumulation, all 32 b)
            ps0 = p0_pool.tile([32, ODF], dt.float32, tag="ps0")
            for g in range(G):
                nc.tensor.matmul(
                    ps0[:],
                    x2d[:, g * 32 : (g + 1) * 32],
                    wr_sb[:, g * ODF : (g + 1) * ODF],
                    start=(g == 0),
                    stop=(g == G - 1),
                )
            # squash on [32, ...]: v0 = fac*s0, fac = ns/((1+ns)sqrt(ns+eps)),
            # s0 = 0.1*T (T = ps0); ns from Square(0.1*T).
            sq0 = spool.tile([32, ODF], dt.float32, tag="sq0")
            nc.scalar.activation(
                out=sq0[:], in_=ps0[:], func=AF.Square, bias=czero[:32], scale=0.1
            )
            ns0 = spool.tile([32, OC], dt.float32, tag="ns0")
            nc.vector.tensor_reduce(
                out=ns0[:],
                in_=sq0[:].rearrange("p (o d) -> p o d", o=OC),
                axis=AX.X,
                op=ALU.add,
            )
            sqn0 = spool.tile([32, OC], dt.float32, tag="sqn0")
            nc.scalar.activation(
                out=sqn0[:], in_=ns0[:], func=AF.Sqrt, bias=ceps[:32]
            )
            den0 = spool.tile([32, OC], dt.float32, tag="den0")
            nc.vector.scalar_tensor_tensor(
                out=den0[:], in0=ns0[:], scalar=1.0, in1=sqn0[:],
                op0=ALU.add, op1=ALU.mult,
            )
            rden0 = spool.tile([32, OC], dt.float32, tag="rden0")
            nc.vector.reciprocal(out=rden0[:], in_=den0[:])
            fac0 = spool.tile([32, OC], dt.float32, tag="fac0")
            # fac = 0.1 * ns * rden  (0.1 for s0 = 0.1*T)
            nc.vector.tensor_tensor(
                out=fac0[:], in0=ns0[:], in1=rden0[:], op=ALU.mult
            )
            nc.vector.tensor_scalar_mul(fac0[:], fac0[:], 0.1)
            v0_bf = spool.tile([32, ODF], dt.bfloat16, tag="v0_bf")
            nc.vector.tensor_tensor(
                out=v0_bf[:].rearrange("p (o d) -> p o d", o=OC),
                in0=ps0[:].rearrange("p (o d) -> p o d", o=OC),
                in1=fac0[:].unsqueeze(2).broadcast_to([32, OC, OD]),
                op=ALU.mult,
            )
            # vrep0 per bgroup via select matmul
            vrep0_t = []
            for bg in range(NBG):
                pv = pv_pool.tile([128, ODF], dt.float32, tag="pv")
                nc.tensor.matmul(
                    pv[:], sel[:, bg * 128 : (bg + 1) * 128], v0_bf[:],
                    start=True, stop=True,
                )
                vr = vpool.tile([128, ODF], dt.bfloat16, tag=f"vrep0{bg}")
                nc.scalar.copy(out=vr[:], in_=pv[:])
                vrep0_t.append(vr)

            # ---- Phase A: einsum for all bgroups
            uh_t = []
            bl_t = []
            for bg in range(NBG):
                uh = uhpool.tile([128, G * ODF], dt.bfloat16, tag=f"uh{bg}")
                uh_t.append(uh)
                bl = blpool.tile([128, GO], dt.float32, tag=f"bl{bg}")
                bl_t.append(bl)

            def einsum_bg(bg):
                uh = uh_t[bg]
                for xc in range(NXCH):
                    xt = xpool.tile([128, XCH * 128], dt.bfloat16, tag="xt")
                    nc.sync.dma_start(
                        out=xt[:],
                        in_=xblk_d[bg][:, xc * XCH * 128 : (xc + 1) * XCH * 128],
                    )
                    for t in range(XCH // 6):
                        pe = pe_pool.tile([128, 960], dt.float32, tag="pe")
                        for j in range(6):
                            gl = t * 6 + j           # local g in chunk
                            g = xc * XCH + gl        # global g
                            nc.tensor.matmul(
                                pe[:, j * ODF : (j + 1) * ODF],
                                xt[:, gl * 128 : (gl + 1) * 128],
                                wr_sb[:, g * ODF : (g + 1) * ODF],
                                start=True,
                                stop=True,
                            )
                        g0 = xc * XCH + t * 6
                        nc.scalar.copy(
                            out=uh[:, g0 * ODF : (g0 + 6) * ODF], in_=pe[:]
                        )

            GPS_TAIL = False  # GpSimd tree tail: measured 4-8x slower than DVE

            def bl_update(bg, vrep, first):
                """bl[bg] (+)= sum_d uh[bg]*vrep ; first=True writes fresh."""
                uh = uh_t[bg]
                bl = bl_t[bg]
                tail = nc.gpsimd if GPS_TAIL else nc.vector
                for ch in range(NZCH):
                    cs = ch * ZCH
                    z = zpool.tile([128, ZCH * ODF], dt.bfloat16, tag="z")
                    nc.vector.tensor_tensor(
                        out=z[:].rearrange("p (g f) -> p g f", f=ODF),
                        in0=uh[:, cs * ODF : (cs + ZCH) * ODF].rearrange(
                            "p (g f) -> p g f", f=ODF
                        ),
                        in1=vrep[:].unsqueeze(1).broadcast_to([128, ZCH, ODF]),
                        op=ALU.mult,
                    )
                    t8 = zpool.tile([128, ZCH * 80], dt.bfloat16, tag="t8")
                    zv = z[:].rearrange("p (g o d) -> p g o d", o=OC, d=OD)
                    nc.vector.tensor_tensor(
                        out=t8[:].rearrange("p (g o d) -> p g o d", o=OC, d=8),
                        in0=zv[:, :, :, 0:8],
                        in1=zv[:, :, :, 8:16],
                        op=ALU.add,
                    )
                    t4 = zpool.tile([128, ZCH * 40], dt.bfloat16, tag="t4")
                    t8v = t8[:].rearrange("p (g o d) -> p g o d", o=OC, d=8)
                    nc.vector.tensor_tensor(
                        out=t4[:].rearrange("p (g o d) -> p g o d", o=OC, d=4),
                        in0=t8v[:, :, :, 0:4],
                        in1=t8v[:, :, :, 4:8],
                        op=ALU.add,
                    )
                    t2 = zpool.tile([128, ZCH * 20], dt.bfloat16, tag="t2")
                    t4v = t4[:].rearrange("p (g o d) -> p g o d", o=OC, d=4)
                    tail.tensor_tensor(
                        out=t2[:].rearrange("p (g o d) -> p g o d", o=OC, d=2),
                        in0=t4v[:, :, :, 0:2],
                        in1=t4v[:, :, :, 2:4],
                        op=ALU.add,
                    )
                    t2v = t2[:].rearrange("p (g o d) -> p g o d", o=OC, d=2)
                    blv = bl[:, cs * OC : (cs + ZCH) * OC]
                    if first:
                        tail.tensor_tensor(
                            out=blv.rearrange("p (g o) -> p g o", o=OC).unsqueeze(3),
                            in0=t2v[:, :, :, 0:1],
                            in1=t2v[:, :, :, 1:2],
                            op=ALU.add,
                        )
                    else:
                        t1 = zpool.tile([128, ZCH * OC], dt.bfloat16, tag="t1")
                        tail.tensor_tensor(
                            out=t1[:].rearrange("p (g o) -> p g o", o=OC).unsqueeze(3),
                            in0=t2v[:, :, :, 0:1],
                            in1=t2v[:, :, :, 1:2],
                            op=ALU.add,
                        )
                        tail.tensor_tensor(
                            out=blv, in0=blv, in1=t1[:], op=ALU.add
                        )

            # ---- iter-0: einsum interleaved with bl updates (software pipeline)
            einsum_bg(0)
            einsum_bg(1)
            bl_update(0, vrep0_t[0], first=True)
            einsum_bg(2)
            bl_update(1, vrep0_t[1], first=True)
            einsum_bg(3)
            bl_update(2, vrep0_t[2], first=True)
            bl_update(3, vrep0_t[3], first=True)

            # ---- iters 1,2: stages, software-pipelined across (it,bg) columns
            col_state = {}

            def stage1(it, bg):
                """softmax over o + call build."""
                bl = bl_t[bg]
                ee = rpool.tile([128, GO], dt.float32, tag="ee")
                nc.scalar.activation(
                    out=ee[:], in_=bl[:], func=AF.Exp, bias=czero[:]
                )
                zz = rpool.tile([128, G], dt.float32, tag="zz")
                nc.vector.tensor_reduce(
                    out=zz[:],
                    in_=ee[:].rearrange("p (g o) -> p g o", o=OC),
                    axis=AX.X,
                    op=ALU.add,
                )
                rz = rpool.tile([128, G], dt.float32, tag="rz")
                nc.vector.reciprocal(out=rz[:], in_=zz[:])
                cCb = rpool.tile([128, GO], dt.bfloat16, tag="cCb")
                nc.vector.tensor_tensor(
                    out=cCb[:].rearrange("p (g o) -> p g o", o=OC),
                    in0=ee[:].rearrange("p (g o) -> p g o", o=OC),
                    in1=rz[:].unsqueeze(2).broadcast_to([128, G, OC]),
                    op=ALU.mult,
                )
                call = rpool.tile([128, G * 80], dt.bfloat16, tag="call")
                callv = call[:].rearrange(
                    "p (g b o) -> p g b o", b=BG, o=OC
                )
                # b' 0:4 on VectorE (one broadcast tensor_tensor)
                nc.vector.tensor_tensor(
                    out=callv[:, :, 0:4, :],
                    in0=cCb[:]
                    .rearrange("p (g o) -> p g o", o=OC)
                    .unsqueeze(2)
                    .broadcast_to([128, G, 4, OC]),
                    in1=mcb[:, 0:40]
                    .rearrange("p (b o) -> p b o", o=OC)
                    .unsqueeze(1)
                    .broadcast_to([128, G, 4, OC]),
                    op=ALU.mult,
                )
                # b' 4:8 on ScalarE (masked copies, per-partition scale)
                for bp in range(4, BG):
                    nc.scalar.activation(
                        out=callv[:, :, bp, :],
                        in_=cCb[:].rearrange("p (g o) -> p g o", o=OC),
                        func=AF.Copy,
                        bias=0.0,
                        scale=mcf[:, bp * OC : bp * OC + 1],
                    )
                col_state[(it, bg)] = {"call": call}

            def stage2(it, bg):
                """s matmul: accumulate over all 72 groups."""
                call = col_state[(it, bg)]["call"]
                uh = uh_t[bg]
                ps = ps_pool.tile([80, ODF], dt.float32, tag="ps")
                for g in range(G):
                    nc.tensor.matmul(
                        ps[:],
                        call[:, g * 80 : (g + 1) * 80],
                        uh[:, g * ODF : (g + 1) * ODF],
                        start=(g == 0),
                        stop=(g == G - 1),
                    )
                col_state[(it, bg)]["ps"] = ps

            def stage3(it, bg):
                """diag extract + squash; it1: build vrep + bl update input,
                it2: final v -> DRAM."""
                ps = col_state[(it, bg)]["ps"]
                # extract diag o==o' -> s_t [80,16]
                tmp = spool.tile([80, ODF], dt.float32, tag="tmp")
                nc.vector.tensor_tensor(
                    out=tmp[:].rearrange("p (d o) -> p d o", o=OC),
                    in0=ps[:]
                    .rearrange("p (o d) -> p o d", o=OC)
                    .transpose([0, 2, 1]),
                    in1=msks[:]
                    .rearrange("p (o d) -> p o d", o=OC)
                    .transpose([0, 2, 1]),
                    op=ALU.mult,
                )
                s_t = spool.tile([80, OD], dt.float32, tag="s_t")
                nc.vector.tensor_reduce(
                    out=s_t[:],
                    in_=tmp[:].rearrange("p (d o) -> p d o", o=OC),
                    axis=AX.X,
                    op=ALU.add,
                )

                # squash: fac = ns / ((1+ns) * sqrt(ns+eps))
                sq = spool.tile([80, OD], dt.float32, tag="sq")
                ns = spool.tile([80, 1], dt.float32, tag="ns")
                nc.scalar.activation(
                    out=sq[:], in_=s_t[:], func=AF.Square, bias=czero[:80]
                )
                nc.vector.tensor_reduce(
                    out=ns[:], in_=sq[:], axis=AX.X, op=ALU.add
                )
                sqn = spool.tile([80, 1], dt.float32, tag="sqn")
                nc.scalar.activation(
                    out=sqn[:], in_=ns[:], func=AF.Sqrt, bias=ceps[:]
                )
                den = spool.tile([80, 1], dt.float32, tag="den")
                nc.vector.scalar_tensor_tensor(
                    out=den[:], in0=ns[:], scalar=1.0, in1=sqn[:],
                    op0=ALU.add, op1=ALU.mult,
                )
                rden = spool.tile([80, 1], dt.float32, tag="rden")
                nc.vector.reciprocal(out=rden[:], in_=den[:])
                fac = spool.tile([80, 1], dt.float32, tag="fac")
                nc.vector.tensor_tensor(
                    out=fac[:], in0=ns[:], in1=rden[:], op=ALU.mult
                )

                if it == ITERS - 1:
                    v_f = spool.tile([80, OD], dt.float32, tag="v_f")
                    nc.vector.tensor_scalar_mul(v_f[:], s_t[:], fac[:])
                    nc.sync.dma_start(
                        out=vout_d[bg * BG : (bg + 1) * BG].rearrange(
                            "b o d -> (b o) d"
                        ),
                        in_=v_f[:],
                    )
                    return

                v_bf = spool.tile([80, OD], dt.bfloat16, tag="v_bf")
                nc.vector.tensor_scalar_mul(v_bf[:], s_t[:], fac[:])

                # vexp[(b,o),(o',d)] = v[b,o,d] * delta(o==o')
                vexp = spool.tile([80, ODF], dt.bfloat16, tag="vexp")
                nc.vector.tensor_tensor(
                    out=vexp[:].rearrange("p (o d) -> p o d", o=OC),
                    in0=msks[:].rearrange("p (o d) -> p o d", o=OC),
                    in1=v_bf[:].unsqueeze(1).broadcast_to([80, OC, OD]),
                    op=ALU.mult,
                )
                pv = pv_pool.tile([128, ODF], dt.float32, tag="pv")
                nc.tensor.matmul(pv[:], arep[:], vexp[:], start=True, stop=True)
                vrep = vpool.tile([128, ODF], dt.bfloat16, tag="vrep")
                nc.scalar.copy(out=vrep[:], in_=pv[:])
                col_state[(it, bg)]["vrep"] = vrep

            # pipelined emission: stage1(n) | stage3(n-1) | stage2(n) |
            # bl_update(n-1, it1 only)
            cols = [(1, 0), (1, 1), (1, 2), (1, 3), (2, 0), (2, 1), (2, 2), (2, 3)]
            for n in range(len(cols)):
                stage1(*cols[n])
                if n >= 1:
                    stage3(*cols[n - 1])
                stage2(*cols[n])
                if n >= 1 and cols[n - 1][0] == 1:
                    bl_update(cols[n - 1][1], col_state[cols[n - 1]]["vrep"],
                              first=False)
            stage3(*cols[-1])

    nc.finalize()
    _BUILT = nc
    return nc


_WARMED = False


def kernel(x, W):
    global _WARMED
    nc = _build()
    in_maps = _in_maps(x, W)
    if not _WARMED:
        # First execution after an in-process compile can return a
        # partially-unwritten output buffer (observed: bgroup 0 rows NaN).
        # Run once with the real inputs and discard.
        run_bass_kernel_spmd(nc, in_maps, core_ids=list(range(NCORES)))
        _WARMED = True
    res = run_bass_kernel_spmd(nc, in_maps, core_ids=list(range(NCORES)))
    outs = res.results
    v = np.concatenate([np.asarray(o["vout"]) for o in outs], axis=0)
    return v.astype(np.float32)


if __name__ == "__main__":
    rng = np.random.default_rng(0)
    x = rng.standard_normal((B, IC, KD), np.float32)
    W = rng.standard_normal((IC, OC, OD, KD), np.float32)
    v = kernel(x, W)
    print("out", v.shape, v.dtype, float(np.abs(v).mean()))
